# revision 1
# baseline (speedup 1.0000x reference)
"""DASGNN aggregator kernel for Trainium2, 8-core SPMD.

Math (per node n, K=32 neighbors, k=16 sampled, D=256):
  st = self_vecs @ Ws                       [N, D]
  l_self = st @ a,  l_j = (x_j @ Wn) @ a = x_j @ (Wn @ a)
  scores = softmax(relu([l_self, l_0..l_31]))
  S = top-16 neighbor scores (ties -> lowest index, matching jax.lax.top_k)
  agg = sum_{j in S} score_j * (x_j @ Wn) = (sum_{j in S} score_j * x_j) @ Wn
  out = relu(st + agg)

The last identity collapses the per-neighbor GEMM (84 GFLOP) into a
weighted reduction of raw neighbor vectors followed by one [N,256]x[256,256]
GEMM, making the kernel memory-bound (neigh_vecs is 655 MB).

Per core (2500 nodes, groups of 256/128/68 nodes; a group of G nodes is
T = G/4 row-tiles of 128 neighbor rows):
  - neighbor rows stream in as [128, 1024] tiles (4 row-tiles per DMA)
  - DVE affine_mul_reduce computes the 128 logit dots per row-tile into a
    collector column; one PE transpose yields [T tiles, 4 nodes x 32 nbrs]
  - ACT relu/exp(+accum for Z), DVE max8/match_replace for exact top-16
    (match_replace picks lowest-index duplicates, matching jax.lax.top_k)
  - weights w = (top16 mask) * exp(l) / Z are transposed back by PE into a
    block-diagonal [128, 4] per row-tile; tiny matmuls (lhsT = x tile) give
    aggT accumulated in PSUM; final GEMM folds agg into the st PSUM bank.

The main loop is software-pipelined: group g's loads + logit dots are issued
before group g-1's softmax/top-k and weighted-sum stages, so the cross-engine
postproc chain hides under the next group's DMA stream. The last two groups
are smaller (128/68 nodes) to shrink the exposed pipeline-drain tail.
"""
import numpy as np

import concourse.bass as bass
import concourse.tile as tile
from concourse import bacc, mybir
from concourse.bass_utils import run_bass_kernel_spmd

FP = mybir.dt.float32
P = 128
D = 256
K = 32
NCORES = 8
NODES_PER_CORE = 2500
# (node_base, n_row_tiles) per group: 9 x 256-node + 128-node + 68-node
GROUPS = [(g * 256, 64) for g in range(9)] + [(2304, 32), (2432, 17)]

_CACHED = {}


def build_kernel():
    nc = bacc.Bacc("TRN2", target_bir_lowering=False, debug=False,
                   enable_asserts=False, num_devices=NCORES)
    xs_d = nc.dram_tensor("xs", [NODES_PER_CORE, D], FP, kind="ExternalInput").ap()
    xn_d = nc.dram_tensor("xn", [NODES_PER_CORE * K, D], FP, kind="ExternalInput").ap()
    wsa_d = nc.dram_tensor("wsa", [P, 2 * 257], FP, kind="ExternalInput").ap()
    wn_d = nc.dram_tensor("wn", [P, 2 * D], FP, kind="ExternalInput").ap()
    wan_d = nc.dram_tensor("wan", [P, D], FP, kind="ExternalInput").ap()
    id_d = nc.dram_tensor("ident", [P, P], FP, kind="ExternalInput").ap()
    out_d = nc.dram_tensor("out", [NODES_PER_CORE, D], FP, kind="ExternalOutput").ap()

    Relu = mybir.ActivationFunctionType.Relu
    Exp = mybir.ActivationFunctionType.Exp

    with tile.TileContext(nc) as tc:
        import contextlib
        ctx = contextlib.ExitStack()
        with ctx:
            const = ctx.enter_context(tc.tile_pool(name="const", bufs=1))
            xpool = ctx.enter_context(tc.tile_pool(name="x", bufs=32))
            xspool = ctx.enter_context(tc.tile_pool(name="xs", bufs=3))
            xstp = ctx.enter_context(tc.tile_pool(name="xst", bufs=3))
            scr = ctx.enter_context(tc.tile_pool(name="scr", bufs=4))
            coll = ctx.enter_context(tc.tile_pool(name="coll", bufs=3))
            small = ctx.enter_context(tc.tile_pool(name="small", bufs=4))
            slot = ctx.enter_context(tc.tile_pool(name="slot", bufs=2))
            bdp = ctx.enter_context(tc.tile_pool(name="bd", bufs=2))
            agp = ctx.enter_context(tc.tile_pool(name="ag", bufs=2))
            outp = ctx.enter_context(tc.tile_pool(name="out", bufs=3))
            pst = ctx.enter_context(tc.tile_pool(name="pst", bufs=2, space="PSUM"))
            pso = ctx.enter_context(tc.tile_pool(name="pso", bufs=4, space="PSUM"))
            psa = ctx.enter_context(tc.tile_pool(name="psa", bufs=2, space="PSUM"))
            dram = ctx.enter_context(tc.tile_pool(name="esd", bufs=2, space="DRAM"))

            wsa = const.tile([P, 2 * 257], FP)
            nc.sync.dma_start(wsa[:], wsa_d)
            wn = const.tile([P, 2 * D], FP)
            nc.sync.dma_start(wn[:], wn_d)
            wan = const.tile([P, D], FP)
            nc.sync.dma_start(wan[:], wan_d)
            ident = const.tile([P, P], FP)
            nc.sync.dma_start(ident[:], id_d)

            def st_sizes(n_nodes):
                out, left = [], n_nodes
                while left > 0:
                    out.append(min(P, left))
                    left -= P
                return out

            def emit_front(gi):
                """Self path + neighbor loads + logit dots for group gi."""
                base, T = GROUPS[gi]
                n_nodes = 4 * T
                state = {"gi": gi}
                # self path: st = xs @ Ws (+ l_self col 256)
                pso_h = []
                es_dram = dram.tile([2, P], FP)
                for h, m in enumerate(st_sizes(n_nodes)):
                    n0 = base + h * P
                    xs_t = xspool.tile([P, D], FP)
                    nc.sync.dma_start(xs_t[0:m, :], xs_d[n0:n0 + m, :])
                    xsT_ps = pst.tile([P, D], FP, tag="tp")
                    for c in range(2):
                        nc.tensor.transpose(xsT_ps[:, c * P:c * P + m],
                                            xs_t[0:m, c * P:(c + 1) * P], ident[0:m, 0:m])
                    xsT = xstp.tile([P, D], FP)
                    nc.scalar.copy(xsT[:], xsT_ps[:])
                    ps = pso.tile([P, 257], FP)
                    pso_h.append((ps, m))
                    for c in range(2):
                        nc.tensor.matmul(ps[0:m, :], xsT[:, c * P:c * P + m],
                                         wsa[:, c * 257:(c + 1) * 257],
                                         start=(c == 0), stop=False)
                    ls = small.tile([P, 1], FP)
                    nc.scalar.activation(ls[0:m, :], ps[0:m, 256:257], Relu)
                    es = small.tile([P, 1], FP)
                    nc.scalar.activation(es[0:m, :], ls[0:m, :], Exp)
                    nc.sync.dma_start(
                        es_dram[h:h + 1, 0:m].rearrange("one p -> p one"), es[0:m, :])
                # es_slot[t, c] = exp(relu(l_self[4t + c]))
                es_slot = slot.tile([64, 4], FP, tag="es")
                nc.sync.dma_start(
                    es_slot[0:T, :],
                    es_dram[:].rearrange("a b -> (a b)")[0:n_nodes]
                    .rearrange("(t c) -> t c", c=4))

                # neighbor loads + logit dots
                collector = coll.tile([P, 64], FP)
                x4s = []
                r_base = base * K // P     # first row-tile index
                for i in range((T + 3) // 4):
                    nt = min(4, T - 4 * i)             # row-tiles this load
                    x4 = xpool.tile([P, 4 * D], FP)
                    x4s.append(x4)
                    r0 = (r_base + 4 * i) * P
                    nc.sync.dma_start(
                        x4[:, 0:nt * D].rearrange("p (f d) -> p f d", f=nt),
                        xn_d[r0:r0 + nt * P, :].rearrange("(f p) d -> p f d", p=P))
                    for r in range(nt):
                        t = 4 * i + r
                        sc = scr.tile([P, D], FP)
                        nc.vector.affine_mul_reduce(
                            out=sc[:], accum_out=collector[:, t:t + 1],
                            in0=x4[:, r * D:(r + 1) * D], in1=wan[:],
                            scale=1.0, bias=0.0)
                state.update(pso_h=pso_h, es_slot=es_slot,
                             collector=collector, x4s=x4s)
                return state

            def emit_back(state):
                """Softmax + top-16 + weighted sums + output for a group."""
                gi = state["gi"]
                base, T = GROUPS[gi]
                pso_h, es_slot = state["pso_h"], state["es_slot"]
                collector, x4s = state["collector"], state["x4s"]

                ct_ps = pst.tile([64, P], FP, tag="tp")
                nc.tensor.transpose(ct_ps[0:T, :], collector[:, 0:T], ident[:])
                r_sb = slot.tile([64, P], FP, tag="r")
                nc.scalar.activation(r_sb[0:T, :], ct_ps[0:T, :], Relu)
                e_sb = slot.tile([64, P], FP, tag="e")
                zn = slot.tile([64, 4], FP, tag="zn")
                for c in range(4):
                    nc.scalar.activation(e_sb[0:T, c * K:(c + 1) * K],
                                         r_sb[0:T, c * K:(c + 1) * K], Exp,
                                         accum_out=zn[0:T, c:c + 1])
                z = slot.tile([64, 4], FP, tag="z")
                nc.vector.tensor_tensor(z[0:T, :], zn[0:T, :], es_slot[0:T, :],
                                        mybir.AluOpType.add)
                rz = slot.tile([64, 4], FP, tag="rz")
                nc.vector.reciprocal(rz[0:T, :], z[0:T, :])
                w_sb = slot.tile([64, P], FP, tag="w")
                for c in range(4):
                    rs = r_sb[0:T, c * K:(c + 1) * K]
                    m8 = small.tile([64, 8], FP)
                    nc.vector.max(m8[0:T, :], rs)
                    r1 = small.tile([64, K], FP)
                    nc.vector.match_replace(r1[0:T, :], m8[0:T, :], rs, -1.0)
                    m8b = small.tile([64, 8], FP)
                    nc.vector.max(m8b[0:T, :], r1[0:T, :])
                    r2 = small.tile([64, K], FP)
                    nc.vector.match_replace(r2[0:T, :], m8b[0:T, :], r1[0:T, :], -1.0)
                    # mw = (r2 < 0) * (1/Z)   [top-16 mask scaled]
                    mw = small.tile([64, K], FP)
                    nc.vector.tensor_scalar(mw[0:T, :], r2[0:T, :], 0.0,
                                            rz[0:T, c:c + 1],
                                            mybir.AluOpType.is_lt,
                                            mybir.AluOpType.mult)
                    nc.vector.tensor_tensor(w_sb[0:T, c * K:(c + 1) * K], mw[0:T, :],
                                            e_sb[0:T, c * K:(c + 1) * K],
                                            mybir.AluOpType.mult)
                # block-diagonal weights: bd[32c+j, 4t+c] = w[node 4t+c, j]
                wT_ps = pst.tile([P, 64], FP, tag="tp")
                nc.tensor.transpose(wT_ps[:, 0:T], w_sb[0:T, :], ident[0:T, 0:T])
                bd = bdp.tile([P, 4 * 64], FP)
                nc.gpsimd.memset(bd[:, 0:4 * T], 0.0)
                bdv = bd[:, 0:4 * T].rearrange("p (t c) -> p t c", c=4)
                for c in range(4):
                    nc.scalar.copy(bdv[c * K:(c + 1) * K, :, c],
                                   wT_ps[c * K:(c + 1) * K, 0:T])

                # weighted raw sums: aggT[f, n]
                ps_ag = psa.tile([P, 2 * D], FP)
                for i in range(len(x4s)):
                    for r in range(min(4, T - 4 * i)):
                        t = 4 * i + r
                        for fc in range(2):
                            nc.tensor.matmul(
                                ps_ag[:, fc * (4 * T) + 4 * t: fc * (4 * T) + 4 * t + 4],
                                x4s[i][:, r * D + fc * P: r * D + (fc + 1) * P],
                                bd[:, 4 * t:4 * t + 4], start=True, stop=True)
                agT = agp.tile([P, 2 * D], FP)
                nc.scalar.copy(agT[:, 0:8 * T], ps_ag[:, 0:8 * T])

                # agg @ Wn folded into st PSUM, relu, store
                for h, (ps, m) in enumerate(pso_h):
                    for fc in range(2):
                        nc.tensor.matmul(
                            ps[0:m, 0:D],
                            agT[:, fc * (4 * T) + h * P: fc * (4 * T) + h * P + m],
                            wn[:, fc * D:(fc + 1) * D],
                            start=False, stop=(fc == 1))
                    ot = outp.tile([P, D], FP)
                    nc.scalar.activation(ot[0:m, :], ps[0:m, 0:D], Relu)
                    n0 = base + h * P
                    nc.sync.dma_start(out_d[n0:n0 + m, :], ot[0:m, :])

            # software pipeline: front(g) runs ahead of back(g-1)
            prev = None
            for gi in range(len(GROUPS)):
                cur = emit_front(gi)
                if prev is not None:
                    emit_back(prev)
                prev = cur
            emit_back(prev)
    nc.compile()
    return nc


def _prep(self_vecs, neigh_vecs, self_weights, neigh_weights, attention_weights):
    wa_s = (self_weights @ attention_weights).astype(np.float32)      # [256,1]
    wa_n = (neigh_weights @ attention_weights).astype(np.float32)
    wsa = np.concatenate([self_weights, wa_s], axis=1)                # [256,257]
    wsa_p = wsa.reshape(2, P, 257).transpose(1, 0, 2).reshape(P, 2 * 257)
    wn_p = neigh_weights.reshape(2, P, D).transpose(1, 0, 2).reshape(P, 2 * D)
    wan_b = np.tile(wa_n.reshape(1, D), (P, 1))
    ident = np.eye(P, dtype=np.float32)
    in_maps = []
    for k in range(NCORES):
        n0 = k * NODES_PER_CORE
        in_maps.append({
            "xs": np.ascontiguousarray(self_vecs[n0:n0 + NODES_PER_CORE]),
            "xn": np.ascontiguousarray(
                neigh_vecs[n0:n0 + NODES_PER_CORE].reshape(NODES_PER_CORE * K, D)),
            "wsa": np.ascontiguousarray(wsa_p.astype(np.float32)),
            "wn": np.ascontiguousarray(wn_p.astype(np.float32)),
            "wan": np.ascontiguousarray(wan_b.astype(np.float32)),
            "ident": ident,
        })
    return in_maps


def kernel(self_vecs, neigh_vecs, self_weights, neigh_weights,
           attention_weights, num_sampled_neighbors, _want_trace=False):
    assert int(num_sampled_neighbors) == 16
    self_vecs = np.asarray(self_vecs, np.float32)
    neigh_vecs = np.asarray(neigh_vecs, np.float32)
    self_weights = np.asarray(self_weights, np.float32)
    neigh_weights = np.asarray(neigh_weights, np.float32)
    attention_weights = np.asarray(attention_weights, np.float32)
    N = self_vecs.shape[0]
    assert N == NODES_PER_CORE * NCORES

    if "nc" not in _CACHED:
        _CACHED["nc"] = build_kernel()
    nc = _CACHED["nc"]
    in_maps = _prep(self_vecs, neigh_vecs, self_weights, neigh_weights,
                    attention_weights)
    res = run_bass_kernel_spmd(nc, in_maps, core_ids=list(range(NCORES)),
                               trace=False)
    out = np.concatenate([res.results[k]["out"] for k in range(NCORES)], axis=0)[:N]
    if _want_trace:
        _CACHED["last_results"] = res
    return out.astype(np.float32)



# revision 28
# speedup vs baseline: 1.2022x; 1.2022x over previous
"""DASGNN aggregator kernel for Trainium2, 8-core SPMD.

Math (per node n, K=32 neighbors, k=16 sampled, D=256):
  st = self_vecs @ Ws                       [N, D]
  l_self = st @ a,  l_j = (x_j @ Wn) @ a = x_j @ (Wn @ a)
  scores = softmax(relu([l_self, l_0..l_31]))
  S = top-16 neighbor scores (ties -> lowest index, matching jax.lax.top_k)
  agg = sum_{j in S} score_j * (x_j @ Wn) = (sum_{j in S} score_j * x_j) @ Wn
  out = relu(st + agg)

The last identity collapses the per-neighbor GEMM into a weighted reduction
of raw neighbor vectors followed by one [N,256]x[256,256] GEMM. The raw
neighbor stream (the only large input) is staged in DRAM as bf16, halving
the HBM traffic; the self/st/fold path stays fp32, so quantization only
touches the attention logits and the weighted neighbor sum (~0.3% rel).

Layout trick: nodes are processed in "q-order" q = 32c + t (t = row-tile
index within a group of <=128 nodes, c = node-within-row-tile). A DVE
32x32 block transpose of the per-row-tile logit columns then lands every
node's 32 neighbor logits in ONE partition row, so softmax + exact top-16
(max8/match_replace, ties -> lowest index) run as single whole-group ops.
The self path loads/stores DRAM rows q-permuted via strided DMA patterns,
so everything else stays aligned with zero extra data movement.

The 256-wide logit dot (a free-axis reduction) is the throughput limiter,
so it is split across three engines per row-tile (mode pattern):
  'a': DVE affine_mul_reduce (one op)
  'p': Pool tensor_tensor mult -> ACT activation-accumulate reduce
  'd': DVE bf16 2x mult       -> ACT activation-accumulate reduce
  'q': Pool tensor_tensor mult -> DVE tensor_reduce
Queues: SP carries only the big streaming loads; ACT carries consts and
output stores, so no small dependent DMA can head-of-line block the stream.
"""
import numpy as np

import concourse.bass as bass
import concourse.tile as tile
from concourse import bacc, mybir
from concourse.bass_utils import run_bass_kernel_spmd

FP = mybir.dt.float32
BF = mybir.dt.bfloat16
P = 128
D = 256
K = 32
NCORES = 8
NODES_PER_CORE = 2500
# (node_base, T row-tiles) per group; G = 4T nodes, G <= 128.
GROUPS = [(g * 128, 32) for g in range(18)] + [(2304, 26), (2408, 23)]

_CACHED = {}
DEBUG = False


_P_TILES = {1, 4, 7, 10, 13, 16, 19, 22, 25, 28}
_D_TILES = set()
_Q_TILES = set()


def dot_modes(T):
    """Engine assignment for each row-tile's logit dot (see module doc)."""
    out = []
    for t in range(T):
        if t in _P_TILES:
            out.append("p")
        elif t in _D_TILES:
            out.append("d")
        elif t in _Q_TILES:
            out.append("q")
        else:
            out.append("a")
    return out


def build_kernel():
    nc = bacc.Bacc("TRN2", target_bir_lowering=False, debug=False,
                   enable_asserts=False, num_devices=NCORES)
    xs_d = nc.dram_tensor("xs", [NODES_PER_CORE, D], FP, kind="ExternalInput").ap()
    xn_d = nc.dram_tensor("xn", [NODES_PER_CORE * K, D], BF, kind="ExternalInput").ap()
    wpk_d = nc.dram_tensor("wpk", [P, 1158], FP, kind="ExternalInput").ap()
    wan_d = nc.dram_tensor("wan", [P, D], BF, kind="ExternalInput").ap()
    out_d = nc.dram_tensor("out", [NODES_PER_CORE, D], FP, kind="ExternalOutput").ap()
    dbg = {}
    if DEBUG:
        for nm, sh, dt in [("xs_t", [P, D], FP), ("es", [P, 1], FP),
                           ("coll", [P, 32], FP), ("lq", [P, 32], FP),
                           ("r2", [P, 32], FP), ("w", [P, 32], FP),
                           ("bd", [P, P], BF), ("agT", [P, 2 * P], FP)]:
            dbg[nm] = nc.dram_tensor("dbg_" + nm, sh, dt,
                                     kind="ExternalOutput").ap()

    Relu = mybir.ActivationFunctionType.Relu
    Exp = mybir.ActivationFunctionType.Exp
    Copy = mybir.ActivationFunctionType.Copy
    add = mybir.AluOpType.add
    mult = mybir.AluOpType.mult
    is_lt = mybir.AluOpType.is_lt
    AX = mybir.AxisListType.X

    with tile.TileContext(nc) as tc:
        import contextlib
        ctx = contextlib.ExitStack()
        with ctx:
            const = ctx.enter_context(tc.tile_pool(name="const", bufs=1))
            xpool = ctx.enter_context(tc.tile_pool(name="x", bufs=16))
            xspool = ctx.enter_context(tc.tile_pool(name="xs", bufs=3))
            xstp = ctx.enter_context(tc.tile_pool(name="xst", bufs=3))
            scr = ctx.enter_context(tc.tile_pool(name="scr", bufs=6))
            prp = ctx.enter_context(tc.tile_pool(name="prp", bufs=28))
            prd = ctx.enter_context(tc.tile_pool(name="prd", bufs=8))
            dmp = ctx.enter_context(tc.tile_pool(name="dmp", bufs=1))
            coll = ctx.enter_context(tc.tile_pool(name="coll", bufs=3))
            small = ctx.enter_context(tc.tile_pool(name="small", bufs=5))
            slot = ctx.enter_context(tc.tile_pool(name="slot", bufs=2))
            bdp = ctx.enter_context(tc.tile_pool(name="bd", bufs=2))
            agp = ctx.enter_context(tc.tile_pool(name="ag", bufs=2))
            outp = ctx.enter_context(tc.tile_pool(name="out", bufs=3))
            pst = ctx.enter_context(tc.tile_pool(name="pst", bufs=2, space="PSUM"))
            pso = ctx.enter_context(tc.tile_pool(name="pso", bufs=3, space="PSUM"))
            psa = ctx.enter_context(tc.tile_pool(name="psa", bufs=2, space="PSUM"))

            wan = const.tile([P, D], BF)
            nc.scalar.dma_start(wan[:], wan_d)
            wpk = const.tile([P, 1158], FP)
            nc.scalar.dma_start(wpk[:], wpk_d)
            wsa_v = wpk[:, 0:514]        # [Ws | Ws@a] packed in 2 row-halves
            wn_v = wpk[:, 514:1026]      # Wn packed in 2 row-halves
            ident = wpk[:, 1026:1154]
            ind4 = wpk[:, 1154:1158]     # ind4[32c+j, c'] = (c == c')

            dump = dmp.tile([P, D], BF)  # ACT-reduce func output, never read

            def emit_dots(state, i):
                """One x8 load + its logit dots (row-tiles 8i..8i+nt)."""
                base, T = GROUPS[state["gi"]]
                collector, modes = state["collector"], state["modes"]
                nt = min(8, T - 8 * i)
                x8 = xpool.tile([P, 8 * D], BF)
                state["x8s"].append(x8)
                r0 = (base * K // P + 8 * i) * P
                nc.sync.dma_start(
                    x8[:, 0:nt * D].rearrange("p (f d) -> p f d", f=nt),
                    xn_d[r0:r0 + nt * P, :].rearrange("(f p) d -> p f d", p=P))
                for r in range(nt):
                    t = 8 * i + r
                    xv = x8[:, r * D:(r + 1) * D]
                    cv = collector[:, t:t + 1]
                    m = modes[t]
                    if m == "a":
                        sc = scr.tile([P, D], BF)
                        nc.vector.affine_mul_reduce(
                            out=sc[:], accum_out=cv, in0=xv, in1=wan[:],
                            scale=1.0, bias=0.0)
                    elif m == "p":
                        pr = prp.tile([P, D], FP)
                        nc.gpsimd.tensor_tensor(pr[:], xv, wan[:], mult)
                        nc.scalar.activation(dump[:], pr[:], Copy, accum_out=cv)
                    else:  # 'd'
                        pr = prd.tile([P, D], BF)
                        nc.vector.tensor_tensor(pr[:], xv, wan[:], mult)
                        nc.scalar.activation(dump[:], pr[:], Copy, accum_out=cv)

            def emit_front1(gi):
                """Self path + first neighbor chunk for group gi."""
                base, T = GROUPS[gi]
                G = 4 * T
                # self path: st = xs @ Ws (+ l_self col 256). Rows are loaded
                # in natural order; the q = 32c+t node permutation happens for
                # free in the st matmul by scanning xsT's node columns in
                # (c, t) order, so the PSUM row q holds node 4t+c.
                xs_t = xspool.tile([P, D], FP)
                if T < 32:
                    nc.gpsimd.memset(xs_t[:], 0.0)
                nc.sync.dma_start(xs_t[0:G, :], xs_d[base:base + G, :])
                xsT_ps = pst.tile([P, D], FP, tag="tp")
                for c in range(2):
                    nc.tensor.transpose(xsT_ps[:, c * P:(c + 1) * P],
                                        xs_t[:, c * P:(c + 1) * P], ident)
                # psum -> SBUF copy, permuting node columns n=4t+c2 into
                # q=32c2+t order (elementwise copy APs may be multi-dim;
                # matmul operand APs may not)
                xsT = xstp.tile([P, D], FP)
                for c in range(2):
                    nc.scalar.copy(
                        xsT[:, c * P:(c + 1) * P]
                        .rearrange("p (c2 t) -> p c2 t", c2=4),
                        xsT_ps[:, c * P:(c + 1) * P]
                        .rearrange("p (t c2) -> p c2 t", c2=4))
                ps = pso.tile([P, 257], FP)
                for c in range(2):
                    nc.tensor.matmul(ps[:, :], xsT[:, c * P:(c + 1) * P],
                                     wsa_v[:, c * 257:(c + 1) * 257],
                                     start=(c == 0), stop=False)
                ls = small.tile([P, 1], FP)
                nc.scalar.activation(ls[:], ps[:, 256:257], Relu)
                es = slot.tile([P, 1], FP, tag="es")
                nc.scalar.activation(es[:], ls[:], Exp)

                collector = coll.tile([P, 32], FP)
                if T < 32:
                    nc.gpsimd.memset(collector[:, T:32], 0.0)
                if DEBUG and gi == 0:
                    nc.scalar.dma_start(dbg["xs_t"], xs_t[:])
                    nc.scalar.dma_start(dbg["es"], es[:])
                state = {"gi": gi, "es": es, "ps": ps, "collector": collector,
                         "x8s": [], "modes": dot_modes(T)}
                emit_dots(state, 0)
                return state

            def emit_front2(state):
                """Remaining neighbor chunks for the group."""
                _, T = GROUPS[state["gi"]]
                for i in range(1, (T + 7) // 8):
                    emit_dots(state, i)

            def emit_back_a(state):
                """Softmax + exact top-16 -> per-node weights (DVE chain)."""
                es = state["es"]
                collector = state["collector"]

                # q-layout logits: row 32c+t = node 4t+c's 32 neighbor logits
                lq = slot.tile([P, 32], FP, tag="lq")
                nc.vector.transpose(lq[:], collector[:])
                r_sb = slot.tile([P, 32], FP, tag="r")
                nc.vector.tensor_scalar_max(r_sb[:], lq[:], 0.0)
                e_sb = slot.tile([P, 32], FP, tag="e")
                zn = slot.tile([P, 1], FP, tag="zn")
                nc.scalar.activation(e_sb[:], r_sb[:], Exp, accum_out=zn[:])
                z = slot.tile([P, 1], FP, tag="z")
                nc.vector.tensor_tensor(z[:], zn[:], es[:], add)
                rz = slot.tile([P, 1], FP, tag="rz")
                nc.vector.reciprocal(rz[:], z[:])
                # exact top-16 (ties -> lowest index, matching jax.lax.top_k)
                m8 = small.tile([P, 8], FP)
                nc.vector.max(m8[:], r_sb[:])
                r1 = small.tile([P, 32], FP)
                nc.vector.match_replace(r1[:], m8[:], r_sb[:], -1.0)
                m8b = small.tile([P, 8], FP)
                nc.vector.max(m8b[:], r1[:])
                r2 = small.tile([P, 32], FP)
                nc.vector.match_replace(r2[:], m8b[:], r1[:], -1.0)
                # w = (top16 mask) * exp(l) / Z
                mw = small.tile([P, 32], FP)
                nc.vector.tensor_scalar(mw[:], r2[:], 0.0, rz[:], is_lt, mult)
                w_sb = slot.tile([P, 32], FP, tag="w")
                nc.vector.tensor_tensor(w_sb[:], mw[:], e_sb[:], mult)
                wT = slot.tile([P, 32], FP, tag="wT")
                nc.vector.transpose(wT[:], w_sb[:])
                state["wT"] = wT
                if DEBUG and state["gi"] == 0:
                    nc.scalar.dma_start(dbg["coll"], state["collector"][:])
                    nc.scalar.dma_start(dbg["lq"], lq[:])
                    nc.scalar.dma_start(dbg["r2"], r2[:])
                    nc.scalar.dma_start(dbg["w"], w_sb[:])

            def emit_back_b(state):
                """Weighted sums + fold GEMM + output for a group."""
                gi = state["gi"]
                base, T = GROUPS[gi]
                G = 4 * T
                ps, x8s, wT = state["ps"], state["x8s"], state["wT"]

                # block-diagonal weights: bd[32c+j, 4t+c] = w[node 4t+c, j].
                # Built as 4 indicator multiplies (bd[:,:,c'] = wT * ind4[:,c'])
                # so the off-diagonal zeros need no separate memset.
                bd = bdp.tile([P, P], BF)
                bdv = bd[:, 0:4 * T].rearrange("p (t c) -> p t c", c=4)
                for c in range(4):
                    nc.gpsimd.tensor_scalar(bdv[:, :, c], wT[:, 0:T],
                                            ind4[:, c:c + 1], None, mult)

                # weighted raw sums: aggT[f, q] accumulated per row-tile
                ps_ag = psa.tile([P, 2 * P], FP)
                agv = [ps_ag[:, fc * P:(fc + 1) * P]
                       .rearrange("p (c t) -> p c t", c=4) for fc in range(2)]
                for i, x8 in enumerate(x8s):
                    for r in range(min(8, T - 8 * i)):
                        t = 8 * i + r
                        for fc in range(2):
                            nc.tensor.matmul(
                                agv[fc][:, :, t],
                                x8[:, r * D + fc * P: r * D + (fc + 1) * P],
                                bd[:, 4 * t:4 * t + 4], start=True, stop=True)
                agT = agp.tile([P, 2 * P], FP)
                nc.scalar.copy(agT[:], ps_ag[:])
                if DEBUG and gi == 0:
                    nc.scalar.dma_start(dbg["bd"], bd[:])
                    nc.scalar.dma_start(dbg["agT"], agT[:])

                # agg @ Wn folded into st PSUM, relu, store (un-permute rows)
                for fc in range(2):
                    nc.tensor.matmul(ps[:, 0:D], agT[:, fc * P:(fc + 1) * P],
                                     wn_v[:, fc * D:(fc + 1) * D],
                                     start=False, stop=(fc == 1))
                ot = outp.tile([P, D], FP)
                nc.scalar.activation(ot[:], ps[:, 0:D], Relu)
                # un-permute rows q=32c+t -> node 4t+c: 4 DMAs, one per c
                # (DRAM-side row striding; SBUF side stays a plain range)
                dv = out_d[base:base + G, :].rearrange("(t c) d -> c t d", c=4)
                for c in range(4):
                    nc.scalar.dma_start(
                        dv[c:c + 1].rearrange("o t d -> (o t) d"),
                        ot[32 * c:32 * c + T, :])

            # software pipeline: the softmax/top-k chain of g-1 is emitted
            # between front(g)'s first and remaining chunks (its deps are
            # ready then), and the weighted-sum/store stage after front(g),
            # so no in-order engine queue ever parks on a far dependency.
            prev = None
            for gi in range(len(GROUPS)):
                cur = emit_front1(gi)
                if prev is not None:
                    emit_back_a(prev)
                emit_front2(cur)
                if prev is not None:
                    emit_back_b(prev)
                prev = cur
            emit_back_a(prev)
            emit_back_b(prev)
    nc.compile()
    return nc


def _prep(self_vecs, neigh_vecs, self_weights, neigh_weights, attention_weights):
    np_bf = mybir.dt.np(BF)
    wa_s = (self_weights @ attention_weights).astype(np.float32)      # [256,1]
    wa_n = (neigh_weights @ attention_weights).astype(np.float32)
    wsa = np.concatenate([self_weights, wa_s], axis=1)                # [256,257]
    wsa_p = wsa.reshape(2, P, 257).transpose(1, 0, 2).reshape(P, 2 * 257)
    wn_p = neigh_weights.reshape(2, P, D).transpose(1, 0, 2).reshape(P, 2 * D)
    ident = np.eye(P, dtype=np.float32)
    ind4 = np.repeat(np.eye(4, dtype=np.float32), K, axis=0)        # [128, 4]
    wpk = np.ascontiguousarray(
        np.concatenate([wsa_p, wn_p, ident, ind4], axis=1).astype(np.float32))
    wan_b = np.ascontiguousarray(
        np.tile(wa_n.reshape(1, D), (P, 1)).astype(np_bf))
    in_maps = []
    for k in range(NCORES):
        n0 = k * NODES_PER_CORE
        in_maps.append({
            "xs": np.ascontiguousarray(self_vecs[n0:n0 + NODES_PER_CORE]),
            "xn": np.ascontiguousarray(
                neigh_vecs[n0:n0 + NODES_PER_CORE]
                .reshape(NODES_PER_CORE * K, D).astype(np_bf)),
            "wpk": wpk,
            "wan": wan_b,
        })
    return in_maps


def kernel(self_vecs, neigh_vecs, self_weights, neigh_weights,
           attention_weights, num_sampled_neighbors, _want_trace=False):
    assert int(num_sampled_neighbors) == 16
    self_vecs = np.asarray(self_vecs, np.float32)
    neigh_vecs = np.asarray(neigh_vecs, np.float32)
    self_weights = np.asarray(self_weights, np.float32)
    neigh_weights = np.asarray(neigh_weights, np.float32)
    attention_weights = np.asarray(attention_weights, np.float32)
    N = self_vecs.shape[0]
    assert N == NODES_PER_CORE * NCORES

    if "nc" not in _CACHED:
        _CACHED["nc"] = build_kernel()
    nc = _CACHED["nc"]
    in_maps = _prep(self_vecs, neigh_vecs, self_weights, neigh_weights,
                    attention_weights)
    res = run_bass_kernel_spmd(nc, in_maps, core_ids=list(range(NCORES)),
                               trace=False)
    out = np.concatenate([res.results[k]["out"] for k in range(NCORES)], axis=0)[:N]
    if _want_trace:
        _CACHED["last_results"] = res
    return out.astype(np.float32)


# revision 39
# speedup vs baseline: 1.4040x; 1.1678x over previous
"""DASGNN aggregator kernel for Trainium2, 8-core SPMD.

Math (per node n, K=32 neighbors, k=16 sampled, D=256):
  st = self_vecs @ Ws                       [N, D]
  l_self = st @ a,  l_j = (x_j @ Wn) @ a = x_j @ (Wn @ a)
  scores = softmax(relu([l_self, l_0..l_31]))
  S = top-16 neighbor scores (ties -> lowest index, matching jax.lax.top_k)
  agg = sum_{j in S} score_j * (x_j @ Wn) = (sum_{j in S} score_j * x_j) @ Wn
  out = relu(st + agg)

The last identity collapses the per-neighbor GEMM into a weighted reduction
of raw neighbor vectors followed by one [N,256]x[256,256] GEMM. The raw
neighbor stream (the only large input) is staged in DRAM as bf16, halving
the HBM traffic; the self/st/fold path stays fp32, so quantization only
touches the attention logits and the weighted neighbor sum (~0.3% rel).

Layout trick: nodes are processed in "q-order" q = 32c + t (t = row-tile
index within a group of <=128 nodes, c = node-within-row-tile). A DVE
32x32 block transpose of the per-row-tile logit columns then lands every
node's 32 neighbor logits in ONE partition row, so softmax + exact top-16
(max8/match_replace, ties -> lowest index) run as single whole-group ops.
The self path loads/stores DRAM rows q-permuted via strided DMA patterns,
so everything else stays aligned with zero extra data movement.

The 256-wide logit dot (a free-axis reduction) is the throughput limiter,
so it is split across three engines per row-tile (mode pattern):
  'a': DVE affine_mul_reduce (one op)
  'p': Pool tensor_tensor mult -> ACT activation-accumulate reduce
  'd': DVE bf16 2x mult       -> ACT activation-accumulate reduce
  'q': Pool tensor_tensor mult -> DVE tensor_reduce
Queues: SP carries only the big streaming loads; ACT carries consts and
output stores, so no small dependent DMA can head-of-line block the stream.
"""
import numpy as np

import concourse.bass as bass
import concourse.tile as tile
from concourse import bacc, mybir
from concourse.bass_utils import run_bass_kernel_spmd

FP = mybir.dt.float32
BF = mybir.dt.bfloat16
P = 128
D = 256
K = 32
NCORES = 8
NODES_PER_CORE = 2500
# (node_base, T row-tiles) per group; G = 4T nodes, G <= 128.
GROUPS = [(g * 128, 32) for g in range(18)] + [(2304, 26), (2408, 23)]

_CACHED = {}
DEBUG = False


_P_TILES = {1, 4, 7, 10, 13, 16, 19, 22, 25, 28}
_D_TILES = set()
_Q_TILES = set()


def dot_modes(T):
    """Engine assignment for each row-tile's logit dot (see module doc)."""
    out = []
    for t in range(T):
        if t in _P_TILES:
            out.append("p")
        elif t in _D_TILES:
            out.append("d")
        elif t in _Q_TILES:
            out.append("q")
        else:
            out.append("a")
    return out


def build_kernel():
    nc = bacc.Bacc("TRN2", target_bir_lowering=False, debug=False,
                   enable_asserts=False, num_devices=NCORES)
    NG = len(GROUPS)
    # xst: host-side pre-transposed, q-permuted, zero-padded self vectors:
    # xst[p, c*128*NG + g*128 + q] = xs_q[g*128+q, 128c+p]
    xst_d = nc.dram_tensor("xst", [P, 2 * P * NG], FP, kind="ExternalInput").ap()
    xn_d = nc.dram_tensor("xn", [NODES_PER_CORE * K, D], BF, kind="ExternalInput").ap()
    wpk_d = nc.dram_tensor("wpk", [P, 1030], FP, kind="ExternalInput").ap()
    wan_d = nc.dram_tensor("wan", [P, D], BF, kind="ExternalInput").ap()
    # output rows stay q-permuted/padded; the host un-permutes after gather
    out_d = nc.dram_tensor("out", [P * NG, D], FP, kind="ExternalOutput").ap()
    dbg = {}
    if DEBUG:
        for nm, sh, dt in [("es", [P, 1], FP),
                           ("coll", [P, 32], FP), ("lq", [P, 32], FP),
                           ("r2", [P, 32], FP), ("w", [P, 32], FP),
                           ("bd", [P, P], BF), ("agT", [P, 2 * P], FP)]:
            dbg[nm] = nc.dram_tensor("dbg_" + nm, sh, dt,
                                     kind="ExternalOutput").ap()

    Relu = mybir.ActivationFunctionType.Relu
    Exp = mybir.ActivationFunctionType.Exp
    Copy = mybir.ActivationFunctionType.Copy
    add = mybir.AluOpType.add
    mult = mybir.AluOpType.mult
    is_lt = mybir.AluOpType.is_lt
    AX = mybir.AxisListType.X

    with tile.TileContext(nc) as tc:
        import contextlib
        ctx = contextlib.ExitStack()
        with ctx:
            const = ctx.enter_context(tc.tile_pool(name="const", bufs=1))
            xpool = ctx.enter_context(tc.tile_pool(name="x", bufs=16))
            xstp = ctx.enter_context(tc.tile_pool(name="xst", bufs=3))
            scr = ctx.enter_context(tc.tile_pool(name="scr", bufs=6))
            prp = ctx.enter_context(tc.tile_pool(name="prp", bufs=28))
            prd = ctx.enter_context(tc.tile_pool(name="prd", bufs=8))
            dmp = ctx.enter_context(tc.tile_pool(name="dmp", bufs=1))
            coll = ctx.enter_context(tc.tile_pool(name="coll", bufs=3))
            small = ctx.enter_context(tc.tile_pool(name="small", bufs=5))
            slot = ctx.enter_context(tc.tile_pool(name="slot", bufs=2))
            bdp = ctx.enter_context(tc.tile_pool(name="bd", bufs=2))
            agp = ctx.enter_context(tc.tile_pool(name="ag", bufs=2))
            outp = ctx.enter_context(tc.tile_pool(name="out", bufs=3))
            pso = ctx.enter_context(tc.tile_pool(name="pso", bufs=3, space="PSUM"))
            psa = ctx.enter_context(tc.tile_pool(name="psa", bufs=2, space="PSUM"))

            wan = const.tile([P, D], BF)
            nc.scalar.dma_start(wan[:], wan_d)
            wpk = const.tile([P, 1030], FP)
            nc.scalar.dma_start(wpk[:], wpk_d)
            wsa_v = wpk[:, 0:514]        # [Ws | Ws@a] packed in 2 row-halves
            wn_v = wpk[:, 514:1026]      # Wn packed in 2 row-halves
            ind4 = wpk[:, 1026:1030]     # ind4[32c+j, c'] = (c == c')

            dump = dmp.tile([P, D], BF)  # ACT-reduce func output, never read

            def emit_dots(state, i):
                """One x8 load + its logit dots (row-tiles 8i..8i+nt)."""
                base, T = GROUPS[state["gi"]]
                collector, modes = state["collector"], state["modes"]
                nt = min(8, T - 8 * i)
                x8 = xpool.tile([P, 8 * D], BF)
                state["x8s"].append(x8)
                r0 = (base * K // P + 8 * i) * P
                nc.sync.dma_start(
                    x8[:, 0:nt * D].rearrange("p (f d) -> p f d", f=nt),
                    xn_d[r0:r0 + nt * P, :].rearrange("(f p) d -> p f d", p=P))
                for r in range(nt):
                    t = 8 * i + r
                    xv = x8[:, r * D:(r + 1) * D]
                    cv = collector[:, t:t + 1]
                    m = modes[t]
                    if m == "a":
                        sc = scr.tile([P, D], BF)
                        nc.vector.affine_mul_reduce(
                            out=sc[:], accum_out=cv, in0=xv, in1=wan[:],
                            scale=1.0, bias=0.0)
                    elif m == "p":
                        pr = prp.tile([P, D], FP)
                        nc.gpsimd.tensor_tensor(pr[:], xv, wan[:], mult)
                        nc.scalar.activation(dump[:], pr[:], Copy, accum_out=cv)
                    else:  # 'd'
                        pr = prd.tile([P, D], BF)
                        nc.vector.tensor_tensor(pr[:], xv, wan[:], mult)
                        nc.scalar.activation(dump[:], pr[:], Copy, accum_out=cv)

            def emit_front1(gi):
                """Self path + first neighbor chunk for group gi."""
                base, T = GROUPS[gi]
                G = 4 * T
                # self path: st = xs @ Ws (+ l_self col 256). xst is already
                # transposed/q-permuted/zero-padded host-side, so the PSUM
                # row q directly holds node 4t+c (zero rows for tail pads).
                xsT = xstp.tile([P, 2 * P], FP)
                nc.sync.dma_start(
                    xsT[:].rearrange("p (c q) -> p c q", c=2),
                    xst_d[:].rearrange("p (c gq) -> p c gq", c=2)
                    [:, :, gi * P:(gi + 1) * P])
                ps = pso.tile([P, 257], FP)
                for c in range(2):
                    nc.tensor.matmul(ps[:, :], xsT[:, c * P:(c + 1) * P],
                                     wsa_v[:, c * 257:(c + 1) * 257],
                                     start=(c == 0), stop=False)
                # es = exp(relu(l_self)) = max(exp(l_self), 1), max folded
                # into the DVE z-add later
                es = slot.tile([P, 1], FP, tag="es")
                nc.scalar.activation(es[:], ps[:, 256:257], Exp)

                collector = coll.tile([P, 32], FP)
                if T < 32:
                    nc.gpsimd.memset(collector[:, T:32], 0.0)
                if DEBUG and gi == 0:
                    nc.scalar.dma_start(dbg["es"], es[:])
                state = {"gi": gi, "es": es, "ps": ps, "collector": collector,
                         "x8s": [], "modes": dot_modes(T)}
                emit_dots(state, 0)
                return state

            def emit_front2(state):
                """Remaining neighbor chunks for the group."""
                _, T = GROUPS[state["gi"]]
                for i in range(1, (T + 7) // 8):
                    emit_dots(state, i)

            def emit_back_a(state):
                """Softmax + exact top-16 -> per-node weights (DVE chain)."""
                es = state["es"]
                collector = state["collector"]

                # q-layout logits: row 32c+t = node 4t+c's 32 neighbor logits
                lq = slot.tile([P, 32], FP, tag="lq")
                nc.vector.transpose(lq[:], collector[:])
                r_sb = slot.tile([P, 32], FP, tag="r")
                nc.vector.tensor_scalar_max(r_sb[:], lq[:], 0.0)
                e_sb = slot.tile([P, 32], FP, tag="e")
                zn = slot.tile([P, 1], FP, tag="zn")
                nc.scalar.activation(e_sb[:], r_sb[:], Exp, accum_out=zn[:])
                z = slot.tile([P, 1], FP, tag="z")
                nc.vector.tensor_scalar(z[:], es[:], 1.0, zn[:],
                                        mybir.AluOpType.max, add)
                rz = slot.tile([P, 1], FP, tag="rz")
                nc.vector.reciprocal(rz[:], z[:])
                # exact top-16 (ties -> lowest index, matching jax.lax.top_k)
                m8 = small.tile([P, 8], FP)
                nc.vector.max(m8[:], r_sb[:])
                r1 = small.tile([P, 32], FP)
                nc.vector.match_replace(r1[:], m8[:], r_sb[:], -1.0)
                m8b = small.tile([P, 8], FP)
                nc.vector.max(m8b[:], r1[:])
                r2 = small.tile([P, 32], FP)
                nc.vector.match_replace(r2[:], m8b[:], r1[:], -1.0)
                # w = (top16 mask) * exp(l) / Z
                mw = small.tile([P, 32], FP)
                nc.vector.tensor_scalar(mw[:], r2[:], 0.0, rz[:], is_lt, mult)
                w_sb = slot.tile([P, 32], FP, tag="w")
                nc.vector.tensor_tensor(w_sb[:], mw[:], e_sb[:], mult)
                wT = slot.tile([P, 32], FP, tag="wT")
                nc.vector.transpose(wT[:], w_sb[:])
                state["wT"] = wT
                if DEBUG and state["gi"] == 0:
                    nc.scalar.dma_start(dbg["coll"], state["collector"][:])
                    nc.scalar.dma_start(dbg["lq"], lq[:])
                    nc.scalar.dma_start(dbg["r2"], r2[:])
                    nc.scalar.dma_start(dbg["w"], w_sb[:])

            def emit_back_b(state):
                """Weighted sums + fold GEMM + output for a group."""
                gi = state["gi"]
                base, T = GROUPS[gi]
                G = 4 * T
                ps, x8s, wT = state["ps"], state["x8s"], state["wT"]

                # block-diagonal weights: bd[32c+j, 4t+c] = w[node 4t+c, j].
                # Built as 4 indicator multiplies (bd[:,:,c'] = wT * ind4[:,c'])
                # so the off-diagonal zeros need no separate memset.
                bd = bdp.tile([P, P], BF)
                bdv = bd[:, 0:4 * T].rearrange("p (t c) -> p t c", c=4)
                for c in range(4):
                    nc.gpsimd.tensor_scalar(bdv[:, :, c], wT[:, 0:T],
                                            ind4[:, c:c + 1], None, mult)

                # weighted raw sums: aggT[f, q] accumulated per row-tile
                ps_ag = psa.tile([P, 2 * P], FP)
                agv = [ps_ag[:, fc * P:(fc + 1) * P]
                       .rearrange("p (c t) -> p c t", c=4) for fc in range(2)]
                for i, x8 in enumerate(x8s):
                    for r in range(min(8, T - 8 * i)):
                        t = 8 * i + r
                        for fc in range(2):
                            nc.tensor.matmul(
                                agv[fc][:, :, t],
                                x8[:, r * D + fc * P: r * D + (fc + 1) * P],
                                bd[:, 4 * t:4 * t + 4], start=True, stop=True)
                agT = agp.tile([P, 2 * P], FP)
                nc.scalar.copy(agT[:], ps_ag[:])
                if DEBUG and gi == 0:
                    nc.scalar.dma_start(dbg["bd"], bd[:])
                    nc.scalar.dma_start(dbg["agT"], agT[:])

                # agg @ Wn folded into st PSUM, relu, store (un-permute rows)
                for fc in range(2):
                    nc.tensor.matmul(ps[:, 0:D], agT[:, fc * P:(fc + 1) * P],
                                     wn_v[:, fc * D:(fc + 1) * D],
                                     start=False, stop=(fc == 1))
                ot = outp.tile([P, D], FP)
                nc.scalar.activation(ot[:], ps[:, 0:D], Relu)
                nc.scalar.dma_start(out_d[gi * P:(gi + 1) * P, :], ot[:])

            # software pipeline: the softmax/top-k chain of g-1 is emitted
            # between front(g)'s first and remaining chunks (its deps are
            # ready then), and the weighted-sum/store stage after front(g),
            # so no in-order engine queue ever parks on a far dependency.
            prev = None
            for gi in range(len(GROUPS)):
                cur = emit_front1(gi)
                if prev is not None:
                    emit_back_a(prev)
                emit_front2(cur)
                if prev is not None:
                    emit_back_b(prev)
                prev = cur
            emit_back_a(prev)
            emit_back_b(prev)
    nc.compile()
    return nc


def _qindex():
    """Storage row g*128 + 32c+t  <->  node base_g + 4t+c (t < T_g)."""
    idx = np.full(P * len(GROUPS), -1, np.int64)
    for g, (b, T) in enumerate(GROUPS):
        for c in range(4):
            for t in range(T):
                idx[g * P + 32 * c + t] = b + 4 * t + c
    return idx, idx >= 0


def _prep(self_vecs, neigh_vecs, self_weights, neigh_weights, attention_weights):
    np_bf = mybir.dt.np(BF)
    NG = len(GROUPS)
    wa_s = (self_weights @ attention_weights).astype(np.float32)      # [256,1]
    wa_n = (neigh_weights @ attention_weights).astype(np.float32)
    wsa = np.concatenate([self_weights, wa_s], axis=1)                # [256,257]
    wsa_p = wsa.reshape(2, P, 257).transpose(1, 0, 2).reshape(P, 2 * 257)
    wn_p = neigh_weights.reshape(2, P, D).transpose(1, 0, 2).reshape(P, 2 * D)
    ind4 = np.repeat(np.eye(4, dtype=np.float32), K, axis=0)        # [128, 4]
    wpk = np.ascontiguousarray(
        np.concatenate([wsa_p, wn_p, ind4], axis=1).astype(np.float32))
    wan_b = np.ascontiguousarray(
        np.tile(wa_n.reshape(1, D), (P, 1)).astype(np_bf))
    idx, valid = _qindex()
    in_maps = []
    for k in range(NCORES):
        n0 = k * NODES_PER_CORE
        xs_q = np.zeros((P * NG, D), np.float32)
        xs_q[valid] = self_vecs[n0:n0 + NODES_PER_CORE][idx[valid]]
        xst = np.ascontiguousarray(
            xs_q.T.reshape(2, P, P * NG).transpose(1, 0, 2)
            .reshape(P, 2 * P * NG))
        in_maps.append({
            "xst": xst,
            "xn": np.ascontiguousarray(
                neigh_vecs[n0:n0 + NODES_PER_CORE]
                .reshape(NODES_PER_CORE * K, D).astype(np_bf)),
            "wpk": wpk,
            "wan": wan_b,
        })
    return in_maps


def kernel(self_vecs, neigh_vecs, self_weights, neigh_weights,
           attention_weights, num_sampled_neighbors, _want_trace=False):
    assert int(num_sampled_neighbors) == 16
    self_vecs = np.asarray(self_vecs, np.float32)
    neigh_vecs = np.asarray(neigh_vecs, np.float32)
    self_weights = np.asarray(self_weights, np.float32)
    neigh_weights = np.asarray(neigh_weights, np.float32)
    attention_weights = np.asarray(attention_weights, np.float32)
    N = self_vecs.shape[0]
    assert N == NODES_PER_CORE * NCORES

    if "nc" not in _CACHED:
        _CACHED["nc"] = build_kernel()
    nc = _CACHED["nc"]
    in_maps = _prep(self_vecs, neigh_vecs, self_weights, neigh_weights,
                    attention_weights)
    res = run_bass_kernel_spmd(nc, in_maps, core_ids=list(range(NCORES)),
                               trace=False)
    idx, valid = _qindex()
    parts = []
    for k in range(NCORES):
        raw = np.asarray(res.results[k]["out"], np.float32)
        out_core = np.empty((NODES_PER_CORE, D), np.float32)
        out_core[idx[valid]] = raw[valid]
        parts.append(out_core)
    out = np.concatenate(parts, axis=0)[:N]
    if _want_trace:
        _CACHED["last_results"] = res
    return out.astype(np.float32)


# revision 42
# speedup vs baseline: 1.4573x; 1.0380x over previous
"""DASGNN aggregator kernel for Trainium2, 8-core SPMD.

Math (per node n, K=32 neighbors, k=16 sampled, D=256):
  st = self_vecs @ Ws                       [N, D]
  l_self = st @ a,  l_j = (x_j @ Wn) @ a = x_j @ (Wn @ a)
  scores = softmax(relu([l_self, l_0..l_31]))
  S = top-16 neighbor scores (ties -> lowest index, matching jax.lax.top_k)
  agg = sum_{j in S} score_j * (x_j @ Wn) = (sum_{j in S} score_j * x_j) @ Wn
  out = relu(st + agg)

The last identity collapses the per-neighbor GEMM into a weighted reduction
of raw neighbor vectors followed by one [N,256]x[256,256] GEMM. The raw
neighbor stream (the only large input) is staged in DRAM as bf16, halving
the HBM traffic; the self/st/fold path stays fp32, so quantization only
touches the attention logits and the weighted neighbor sum (~0.3% rel).

Layout trick: nodes are processed in "q-order" q = 32c + t (t = row-tile
index within a group of <=128 nodes, c = node-within-row-tile). A DVE
32x32 block transpose of the per-row-tile logit columns then lands every
node's 32 neighbor logits in ONE partition row, so softmax + exact top-16
(max8/match_replace, ties -> lowest index) run as single whole-group ops.
The self path loads/stores DRAM rows q-permuted via strided DMA patterns,
so everything else stays aligned with zero extra data movement.

The 256-wide logit dot (a free-axis reduction) is the throughput limiter,
so it is split across three engines per row-tile (mode pattern):
  'a': DVE affine_mul_reduce (one op)
  'p': Pool tensor_tensor mult -> ACT activation-accumulate reduce
  'd': DVE bf16 2x mult       -> ACT activation-accumulate reduce
  'q': Pool tensor_tensor mult -> DVE tensor_reduce
Queues: SP carries only the big streaming loads; ACT carries consts and
output stores, so no small dependent DMA can head-of-line block the stream.
"""
import numpy as np

import concourse.bass as bass
import concourse.tile as tile
from concourse import bacc, mybir
from concourse.bass_utils import run_bass_kernel_spmd

FP = mybir.dt.float32
BF = mybir.dt.bfloat16
P = 128
D = 256
K = 32
NCORES = 8
NODES_PER_CORE = 2500
# (node_base, T row-tiles) per group; G = 4T nodes, G <= 128.
GROUPS = [(g * 128, 32) for g in range(18)] + [(2304, 26), (2408, 23)]

_CACHED = {}
DEBUG = False


_P_TILES = {1, 4, 7, 10, 13, 16, 19, 22, 25, 28}
_D_TILES = set()
_Q_TILES = set()


def dot_modes(T):
    """Engine assignment for each row-tile's logit dot (see module doc)."""
    out = []
    for t in range(T):
        if t in _P_TILES:
            out.append("p")
        elif t in _D_TILES:
            out.append("d")
        elif t in _Q_TILES:
            out.append("q")
        else:
            out.append("a")
    return out


def build_kernel():
    nc = bacc.Bacc("TRN2", target_bir_lowering=False, debug=False,
                   enable_asserts=False, num_devices=NCORES)
    NG = len(GROUPS)
    # xst: host-side pre-transposed, q-permuted, zero-padded self vectors:
    # xst[p, c*128*NG + g*128 + q] = xs_q[g*128+q, 128c+p]
    xst_d = nc.dram_tensor("xst", [P, 2 * P * NG], FP, kind="ExternalInput").ap()
    xn_d = nc.dram_tensor("xn", [NODES_PER_CORE * K, D], BF, kind="ExternalInput").ap()
    wpk_d = nc.dram_tensor("wpk", [P, 1030], FP, kind="ExternalInput").ap()
    wan_d = nc.dram_tensor("wan", [P, D], BF, kind="ExternalInput").ap()
    # output rows stay q-permuted/padded; the host un-permutes after gather
    out_d = nc.dram_tensor("out", [P * NG, D], FP, kind="ExternalOutput").ap()
    dbg = {}
    if DEBUG:
        for nm, sh, dt in [("es", [P, 1], FP),
                           ("coll", [P, 32], FP), ("lq", [P, 32], FP),
                           ("r2", [P, 32], FP), ("w", [P, 32], FP),
                           ("bd", [P, P], BF), ("agT", [P, 2 * P], FP)]:
            dbg[nm] = nc.dram_tensor("dbg_" + nm, sh, dt,
                                     kind="ExternalOutput").ap()

    Relu = mybir.ActivationFunctionType.Relu
    Exp = mybir.ActivationFunctionType.Exp
    Copy = mybir.ActivationFunctionType.Copy
    add = mybir.AluOpType.add
    mult = mybir.AluOpType.mult
    is_lt = mybir.AluOpType.is_lt
    AX = mybir.AxisListType.X

    with tile.TileContext(nc) as tc:
        import contextlib
        ctx = contextlib.ExitStack()
        with ctx:
            const = ctx.enter_context(tc.tile_pool(name="const", bufs=1))
            xpool = ctx.enter_context(tc.tile_pool(name="x", bufs=20))
            xstp = ctx.enter_context(tc.tile_pool(name="xst", bufs=4))
            scr = ctx.enter_context(tc.tile_pool(name="scr", bufs=8))
            prp = ctx.enter_context(tc.tile_pool(name="prp", bufs=34))
            prd = ctx.enter_context(tc.tile_pool(name="prd", bufs=8))
            dmp = ctx.enter_context(tc.tile_pool(name="dmp", bufs=1))
            coll = ctx.enter_context(tc.tile_pool(name="coll", bufs=3))
            small = ctx.enter_context(tc.tile_pool(name="small", bufs=5))
            slot = ctx.enter_context(tc.tile_pool(name="slot", bufs=2))
            bdp = ctx.enter_context(tc.tile_pool(name="bd", bufs=2))
            agp = ctx.enter_context(tc.tile_pool(name="ag", bufs=2))
            outp = ctx.enter_context(tc.tile_pool(name="out", bufs=3))
            pso = ctx.enter_context(tc.tile_pool(name="pso", bufs=3, space="PSUM"))
            psa = ctx.enter_context(tc.tile_pool(name="psa", bufs=2, space="PSUM"))

            wan = const.tile([P, D], BF)
            nc.scalar.dma_start(wan[:], wan_d)
            wpk = const.tile([P, 1030], FP)
            nc.scalar.dma_start(wpk[:], wpk_d)
            wsa_v = wpk[:, 0:514]        # [Ws | Ws@a] packed in 2 row-halves
            wn_v = wpk[:, 514:1026]      # Wn packed in 2 row-halves
            ind4 = wpk[:, 1026:1030]     # ind4[32c+j, c'] = (c == c')

            dump = dmp.tile([P, D], BF)  # ACT-reduce func output, never read

            def emit_dots(state, i):
                """One x8 load + its logit dots (row-tiles 8i..8i+nt)."""
                base, T = GROUPS[state["gi"]]
                collector, modes = state["collector"], state["modes"]
                nt = min(8, T - 8 * i)
                x8 = xpool.tile([P, 8 * D], BF)
                state["x8s"].append(x8)
                r0 = (base * K // P + 8 * i) * P
                nc.sync.dma_start(
                    x8[:, 0:nt * D].rearrange("p (f d) -> p f d", f=nt),
                    xn_d[r0:r0 + nt * P, :].rearrange("(f p) d -> p f d", p=P))
                for r in range(nt):
                    t = 8 * i + r
                    xv = x8[:, r * D:(r + 1) * D]
                    cv = collector[:, t:t + 1]
                    m = modes[t]
                    if m == "a":
                        sc = scr.tile([P, D], BF)
                        nc.vector.affine_mul_reduce(
                            out=sc[:], accum_out=cv, in0=xv, in1=wan[:],
                            scale=1.0, bias=0.0)
                    elif m == "p":
                        pr = prp.tile([P, D], FP)
                        nc.gpsimd.tensor_tensor(pr[:], xv, wan[:], mult)
                        nc.scalar.activation(dump[:], pr[:], Copy, accum_out=cv)
                    else:  # 'd'
                        pr = prd.tile([P, D], BF)
                        nc.vector.tensor_tensor(pr[:], xv, wan[:], mult)
                        nc.scalar.activation(dump[:], pr[:], Copy, accum_out=cv)

            def emit_front1(gi):
                """Self path + first neighbor chunk for group gi."""
                base, T = GROUPS[gi]
                G = 4 * T
                # self path: st = xs @ Ws (+ l_self col 256). xst is already
                # transposed/q-permuted/zero-padded host-side, so the PSUM
                # row q directly holds node 4t+c (zero rows for tail pads).
                xsT = xstp.tile([P, 2 * P], FP)
                nc.sync.dma_start(
                    xsT[:].rearrange("p (c q) -> p c q", c=2),
                    xst_d[:].rearrange("p (c gq) -> p c gq", c=2)
                    [:, :, gi * P:(gi + 1) * P])
                ps = pso.tile([P, 257], FP)
                for c in range(2):
                    nc.tensor.matmul(ps[:, :], xsT[:, c * P:(c + 1) * P],
                                     wsa_v[:, c * 257:(c + 1) * 257],
                                     start=(c == 0), stop=False)
                # es = exp(relu(l_self)) = max(exp(l_self), 1), max folded
                # into the DVE z-add later
                es = slot.tile([P, 1], FP, tag="es")
                nc.scalar.activation(es[:], ps[:, 256:257], Exp)

                collector = coll.tile([P, 32], FP)
                if T < 32:
                    nc.gpsimd.memset(collector[:, T:32], 0.0)
                if DEBUG and gi == 0:
                    nc.scalar.dma_start(dbg["es"], es[:])
                state = {"gi": gi, "es": es, "ps": ps, "collector": collector,
                         "x8s": [], "modes": dot_modes(T)}
                emit_dots(state, 0)
                return state

            def emit_front2(state):
                """Remaining neighbor chunks for the group."""
                _, T = GROUPS[state["gi"]]
                for i in range(1, (T + 7) // 8):
                    emit_dots(state, i)

            def emit_back_a(state):
                """Softmax + exact top-16 -> per-node weights (DVE chain)."""
                es = state["es"]
                collector = state["collector"]

                # q-layout logits: row 32c+t = node 4t+c's 32 neighbor logits
                lq = slot.tile([P, 32], FP, tag="lq")
                nc.vector.transpose(lq[:], collector[:])
                r_sb = slot.tile([P, 32], FP, tag="r")
                nc.vector.tensor_scalar_max(r_sb[:], lq[:], 0.0)
                e_sb = slot.tile([P, 32], FP, tag="e")
                zn = slot.tile([P, 1], FP, tag="zn")
                nc.scalar.activation(e_sb[:], r_sb[:], Exp, accum_out=zn[:])
                z = slot.tile([P, 1], FP, tag="z")
                nc.vector.tensor_scalar(z[:], es[:], 1.0, zn[:],
                                        mybir.AluOpType.max, add)
                rz = slot.tile([P, 1], FP, tag="rz")
                nc.vector.reciprocal(rz[:], z[:])
                # exact top-16 (ties -> lowest index, matching jax.lax.top_k)
                m8 = small.tile([P, 8], FP)
                nc.vector.max(m8[:], r_sb[:])
                r1 = small.tile([P, 32], FP)
                nc.vector.match_replace(r1[:], m8[:], r_sb[:], -1.0)
                m8b = small.tile([P, 8], FP)
                nc.vector.max(m8b[:], r1[:])
                r2 = small.tile([P, 32], FP)
                nc.vector.match_replace(r2[:], m8b[:], r1[:], -1.0)
                # w = (top16 mask) * exp(l) / Z
                mw = small.tile([P, 32], FP)
                nc.vector.tensor_scalar(mw[:], r2[:], 0.0, rz[:], is_lt, mult)
                w_sb = slot.tile([P, 32], FP, tag="w")
                nc.vector.tensor_tensor(w_sb[:], mw[:], e_sb[:], mult)
                wT = slot.tile([P, 32], FP, tag="wT")
                nc.vector.transpose(wT[:], w_sb[:])
                state["wT"] = wT
                if DEBUG and state["gi"] == 0:
                    nc.scalar.dma_start(dbg["coll"], state["collector"][:])
                    nc.scalar.dma_start(dbg["lq"], lq[:])
                    nc.scalar.dma_start(dbg["r2"], r2[:])
                    nc.scalar.dma_start(dbg["w"], w_sb[:])

            def emit_back_b(state):
                """Weighted sums + fold GEMM + output for a group."""
                gi = state["gi"]
                base, T = GROUPS[gi]
                G = 4 * T
                ps, x8s, wT = state["ps"], state["x8s"], state["wT"]

                # block-diagonal weights: bd[32c+j, 4t+c] = w[node 4t+c, j].
                # Built as 4 indicator multiplies (bd[:,:,c'] = wT * ind4[:,c'])
                # so the off-diagonal zeros need no separate memset.
                bd = bdp.tile([P, P], BF)
                bdv = bd[:, 0:4 * T].rearrange("p (t c) -> p t c", c=4)
                for c in range(4):
                    nc.gpsimd.tensor_scalar(bdv[:, :, c], wT[:, 0:T],
                                            ind4[:, c:c + 1], None, mult)

                # weighted raw sums: aggT[f, q] accumulated per row-tile
                ps_ag = psa.tile([P, 2 * P], FP)
                agv = [ps_ag[:, fc * P:(fc + 1) * P]
                       .rearrange("p (c t) -> p c t", c=4) for fc in range(2)]
                for i, x8 in enumerate(x8s):
                    for r in range(min(8, T - 8 * i)):
                        t = 8 * i + r
                        for fc in range(2):
                            nc.tensor.matmul(
                                agv[fc][:, :, t],
                                x8[:, r * D + fc * P: r * D + (fc + 1) * P],
                                bd[:, 4 * t:4 * t + 4], start=True, stop=True)
                agT = agp.tile([P, 2 * P], FP)
                nc.scalar.copy(agT[:], ps_ag[:])
                if DEBUG and gi == 0:
                    nc.scalar.dma_start(dbg["bd"], bd[:])
                    nc.scalar.dma_start(dbg["agT"], agT[:])

                # agg @ Wn folded into st PSUM, relu, store (un-permute rows)
                for fc in range(2):
                    nc.tensor.matmul(ps[:, 0:D], agT[:, fc * P:(fc + 1) * P],
                                     wn_v[:, fc * D:(fc + 1) * D],
                                     start=False, stop=(fc == 1))
                ot = outp.tile([P, D], FP)
                nc.scalar.activation(ot[:], ps[:, 0:D], Relu)
                nc.scalar.dma_start(out_d[gi * P:(gi + 1) * P, :], ot[:])

            # software pipeline: the softmax/top-k chain of g-1 is emitted
            # between front(g)'s first and remaining chunks (its deps are
            # ready then), and the weighted-sum/store stage after front(g),
            # so no in-order engine queue ever parks on a far dependency.
            prev = None
            for gi in range(len(GROUPS)):
                cur = emit_front1(gi)
                if prev is not None:
                    emit_back_a(prev)
                emit_front2(cur)
                if prev is not None:
                    emit_back_b(prev)
                prev = cur
            emit_back_a(prev)
            emit_back_b(prev)
    nc.compile()
    return nc


def _qindex():
    """Storage row g*128 + 32c+t  <->  node base_g + 4t+c (t < T_g)."""
    idx = np.full(P * len(GROUPS), -1, np.int64)
    for g, (b, T) in enumerate(GROUPS):
        for c in range(4):
            for t in range(T):
                idx[g * P + 32 * c + t] = b + 4 * t + c
    return idx, idx >= 0


def _prep(self_vecs, neigh_vecs, self_weights, neigh_weights, attention_weights):
    np_bf = mybir.dt.np(BF)
    NG = len(GROUPS)
    wa_s = (self_weights @ attention_weights).astype(np.float32)      # [256,1]
    wa_n = (neigh_weights @ attention_weights).astype(np.float32)
    wsa = np.concatenate([self_weights, wa_s], axis=1)                # [256,257]
    wsa_p = wsa.reshape(2, P, 257).transpose(1, 0, 2).reshape(P, 2 * 257)
    wn_p = neigh_weights.reshape(2, P, D).transpose(1, 0, 2).reshape(P, 2 * D)
    ind4 = np.repeat(np.eye(4, dtype=np.float32), K, axis=0)        # [128, 4]
    wpk = np.ascontiguousarray(
        np.concatenate([wsa_p, wn_p, ind4], axis=1).astype(np.float32))
    wan_b = np.ascontiguousarray(
        np.tile(wa_n.reshape(1, D), (P, 1)).astype(np_bf))
    idx, valid = _qindex()
    in_maps = []
    for k in range(NCORES):
        n0 = k * NODES_PER_CORE
        xs_q = np.zeros((P * NG, D), np.float32)
        xs_q[valid] = self_vecs[n0:n0 + NODES_PER_CORE][idx[valid]]
        xst = np.ascontiguousarray(
            xs_q.T.reshape(2, P, P * NG).transpose(1, 0, 2)
            .reshape(P, 2 * P * NG))
        in_maps.append({
            "xst": xst,
            "xn": np.ascontiguousarray(
                neigh_vecs[n0:n0 + NODES_PER_CORE]
                .reshape(NODES_PER_CORE * K, D).astype(np_bf)),
            "wpk": wpk,
            "wan": wan_b,
        })
    return in_maps


def kernel(self_vecs, neigh_vecs, self_weights, neigh_weights,
           attention_weights, num_sampled_neighbors, _want_trace=False):
    assert int(num_sampled_neighbors) == 16
    self_vecs = np.asarray(self_vecs, np.float32)
    neigh_vecs = np.asarray(neigh_vecs, np.float32)
    self_weights = np.asarray(self_weights, np.float32)
    neigh_weights = np.asarray(neigh_weights, np.float32)
    attention_weights = np.asarray(attention_weights, np.float32)
    N = self_vecs.shape[0]
    assert N == NODES_PER_CORE * NCORES

    if "nc" not in _CACHED:
        _CACHED["nc"] = build_kernel()
    nc = _CACHED["nc"]
    in_maps = _prep(self_vecs, neigh_vecs, self_weights, neigh_weights,
                    attention_weights)
    res = run_bass_kernel_spmd(nc, in_maps, core_ids=list(range(NCORES)),
                               trace=False)
    idx, valid = _qindex()
    parts = []
    for k in range(NCORES):
        raw = np.asarray(res.results[k]["out"], np.float32)
        out_core = np.empty((NODES_PER_CORE, D), np.float32)
        out_core[idx[valid]] = raw[valid]
        parts.append(out_core)
    out = np.concatenate(parts, axis=0)[:N]
    if _want_trace:
        _CACHED["last_results"] = res
    return out.astype(np.float32)
